# revision 1
# baseline (speedup 1.0000x reference)
"""Trainium2 Bass kernel for MockMobGatedDeltaNetMoE (v8).

Sharding: head-parallel over H=8 heads, one head per NeuronCore.
Each core computes its head's full contribution; the host sums the 8
partial output projections.

Pipeline (all matmuls fp16 operands, fp32 PSUM accumulation):
 - Hidden states ship ONLY as an exact bf16 hi/lo pair (the router needs
   exact fp32 logits for stable top-2); fp16 hs is derived on-device (hi+lo).
 - Phase 1: weights loaded once; q and k projected directly into transposed
   [d, token] layout (weight-stationary); v/g share one hs-stationary pass.
 - Router: exact 3-term bf16 split GEMM -> top-2 routing on DVE.
 - silu(g) precomputed in-place (one sigmoid table load, before the exp
   stream starts).
 - Score fusion: S_r = q @ (Wq_exp_r @ Wk_exp_r^T) @ k^T, M_r fused on host.
 - Key masks are broadcast to [128, TB] tiles via rank-1 PE matmuls
   (ones x maskrow) and applied to kT with one fp16 DVE multiply per chunk;
   masked keys give exp(0)=1 = the reference denominator.
 - Scores computed transposed [query, key]; exp emits the denominator via
   accum_out.
 - Expert combine in PSUM via diagonal matmuls (transpose + scale +
   accumulate per (expert, key-tile)); one attention @ V matmul per query
   tile + rank-4 masked-v correction.
 - Phase 4: gate with precomputed silu, PE transpose, Wo projection, fp16
   partial output (host sums in fp32).
"""

import numpy as np

import concourse.bass as bass
import concourse.bacc as bacc
import concourse.tile as tile
from concourse import mybir
from concourse.bass_utils import run_bass_kernel_spmd

F32 = mybir.dt.float32
F16 = mybir.dt.float16
BF16 = mybir.dt.bfloat16
ALU = mybir.AluOpType
ACTF = mybir.ActivationFunctionType
AX = mybir.AxisListType

H, D, R, NE = 8, 256, 6, 4
HID, DV, T = 2048, 512, 2048
NB = 2
TB = T // NB
SCALE = 1.0 / 16.0


def _body(ctx, nc, tc, io):
    wq, wk, wv, wg, wqm, hsh, hsl, wfh, wfl, wo, out = io

    const = ctx.enter_context(tc.tile_pool(name="const", bufs=1))
    pers = ctx.enter_context(tc.tile_pool(name="pers", bufs=1))

    from concourse.masks import make_identity
    ident = const.tile([128, 128], F32, name="ident")
    make_identity(nc, ident)
    ident16 = const.tile([128, 128], F16, name="ident16")
    nc.vector.tensor_copy(ident16[:], ident[:])
    ones16 = const.tile([128, 128], F16, name="ones16")
    nc.vector.memset(ones16[:], 1.0)
    wfh_sb = const.tile([128, 64], BF16, name="wfh_sb")
    wfl_sb = const.tile([128, 64], BF16, name="wfl_sb")
    for hc in range(16):
        nc.sync.dma_start(out=wfh_sb[:, hc * 4:(hc + 1) * 4],
                          in_=wfh[hc * 128:(hc + 1) * 128, :])
        nc.sync.dma_start(out=wfl_sb[:, hc * 4:(hc + 1) * 4],
                          in_=wfl[hc * 128:(hc + 1) * 128, :])
    logit_sb = pers.tile([128, 64], F32, name="logit_sb")

    qT = pers.tile([128, 2 * T], F16, name="qT")         # [d-chunk, token]
    kT = pers.tile([128, 2 * T], F16, name="kT")         # [d-chunk, token]
    v_sb = pers.tile([128, 16 * DV], F16, name="v_sb")   # [token-tile, dv]
    g_sb = pers.tile([128, 16 * DV], F16, name="g_sb")   # [token-tile, dv]
    wqm_sb = pers.tile([128, 2 * 1536], F16, name="wqm_sb")
    rw_all = pers.tile([128, 16 * R], F32, name="rw_all")
    msel = pers.tile([128, 16 * NE], F16, name="msel")   # top-2 mask (1/0)
    nsel = pers.tile([128, 16 * NE], F16, name="nsel")   # sel - 1 (0/-1)
    wo_sb = [pers.tile([128, HID], F16, name=f"wo_sb{i}") for i in range(4)]
    for dc in range(2):
        nc.sync.dma_start(out=wqm_sb[:, dc * 1536:(dc + 1) * 1536],
                          in_=wqm[dc * 128:(dc + 1) * 128, :])

    # ---------------- phase 1: projections (weights loaded once) ----------------
    with tc.tile_pool(name="p1w", bufs=1) as p1w, \
         tc.tile_pool(name="p1", bufs=1) as p1, \
         tc.tile_pool(name="p1ps", bufs=1, space="PSUM") as p1ps:
        # interleave hidden-state and weight DMAs per hid-chunk so the first
        # matmul can start after a couple of transfers.
        wq_sb, wk_sb, wv_sb, wg_sb = [], [], [], []
        hih0, hil0, hst0 = [], [], []
        for hc in range(16):
            h1 = p1.tile([128, 512], BF16, name="hih", tag="hih", bufs=17)
            nc.sync.dma_start(out=h1[:], in_=hsh[hc * 128:(hc + 1) * 128, 0:512])
            hih0.append(h1)
            h2 = p1.tile([128, 512], BF16, name="hil", tag="hil", bufs=17)
            nc.sync.dma_start(out=h2[:], in_=hsl[hc * 128:(hc + 1) * 128, 0:512])
            hil0.append(h2)
            h3 = p1.tile([128, 512], F16, name="hst", tag="hst", bufs=17)
            nc.vector.tensor_tensor(h3[:], h1[:], h2[:], ALU.add)  # fp16 hs
            hst0.append(h3)
            for lst, src, wdt, nm in ((wq_sb, wq, 256, "wqsb"), (wk_sb, wk, 256, "wksb"),
                                      (wv_sb, wv, 512, "wvsb"), (wg_sb, wg, 512, "wgsb")):
                w1 = p1w.tile([128, wdt], F16, name=nm, tag=f"{nm}{hc}")
                nc.sync.dma_start(out=w1[:], in_=src[hc * 128:(hc + 1) * 128, :])
                lst.append(w1)
        for i in range(4):
            nc.sync.dma_start(out=wo_sb[i][:], in_=wo[i * 128:(i + 1) * 128, :])
        for tb in range(4):  # token blocks of 512
            t0 = tb * 512
            if tb == 0:
                hih, hil, hst = hih0, hil0, hst0
            else:
                hih, hil, hst = [], [], []
                for hc in range(16):
                    h1 = p1.tile([128, 512], BF16, name="hih", tag="hih", bufs=17)
                    nc.sync.dma_start(out=h1[:], in_=hsh[hc * 128:(hc + 1) * 128, t0:t0 + 512])
                    hih.append(h1)
                    h2 = p1.tile([128, 512], BF16, name="hil", tag="hil", bufs=17)
                    nc.sync.dma_start(out=h2[:], in_=hsl[hc * 128:(hc + 1) * 128, t0:t0 + 512])
                    hil.append(h2)
                    h3 = p1.tile([128, 512], F16, name="hst", tag="hst", bufs=17)
                    nc.vector.tensor_tensor(h3[:], h1[:], h2[:], ALU.add)
                    hst.append(h3)
            # q/k projections -> transposed [d-chunk, token] (weight-stationary)
            for wsb, dstT in ((wq_sb, qT), (wk_sb, kT)):
                for f in range(2):
                    ps = p1ps.tile([128, 512], F32, name="psq", tag="psq", bufs=2)
                    for hc in range(16):
                        nc.tensor.matmul(ps[:], wsb[hc][:, f * 128:(f + 1) * 128],
                                         hst[hc][:], start=(hc == 0), stop=(hc == 15))
                    nc.scalar.copy(dstT[:, f * T + t0:f * T + t0 + 512], ps[:])
            # v/g: one shared-stationary pass per token tile
            for tt4 in range(4):
                tt = tb * 4 + tt4
                ps = p1ps.tile([128, 1024], F32, name="psvg", tag="psvg", bufs=2)
                for hc in range(16):
                    st_ap = hst[hc][:, tt4 * 128:(tt4 + 1) * 128]
                    nc.tensor.matmul(ps[:, 0:512], st_ap, wv_sb[hc][:],
                                     start=(hc == 0), stop=(hc == 15))
                    nc.tensor.matmul(ps[:, 512:1024], st_ap, wg_sb[hc][:],
                                     start=(hc == 0), stop=(hc == 15))
                nc.scalar.copy(v_sb[:, tt * DV:(tt + 1) * DV], ps[:, 0:512])
                nc.vector.tensor_copy(g_sb[:, tt * DV:(tt + 1) * DV], ps[:, 512:1024])
            # routing logits: 3-term bf16 split-GEMM (exact products, fp32 accum)
            for tl in range(4):
                tt = tb * 4 + tl
                psr = p1ps.tile([128, 4], F32, name="psr", tag="psq", bufs=2)
                n_mm = 0
                for aa, bb in ((hih, wfh_sb), (hih, wfl_sb), (hil, wfh_sb)):
                    for hc in range(16):
                        nc.tensor.matmul(psr[:],
                                         aa[hc][:, tl * 128:(tl + 1) * 128],
                                         bb[:, hc * 4:(hc + 1) * 4],
                                         start=(n_mm == 0), stop=(n_mm == 47))
                        n_mm += 1
                nc.scalar.copy(logit_sb[:, tt * 4:(tt + 1) * 4], psr[:])

    # ---------------- phase 2: routing ----------------
    nc.vector.memset(rw_all[:], 0.25)
    with tc.tile_pool(name="p2", bufs=4) as p2:
        for tt in range(16):
            lg = logit_sb[:, tt * 4:(tt + 1) * 4]
            s = p2.tile([128, 4], F32, name="s")
            nc.scalar.activation(s[:], lg, ACTF.Exp)
            m1 = p2.tile([128, 1], F32, name="m1")
            nc.vector.tensor_reduce(m1[:], lg, axis=AX.X, op=ALU.max)
            eq = p2.tile([128, 4], F32, name="eq")
            nc.vector.tensor_scalar(eq[:], lg, m1[:], None, ALU.is_ge)
            sm = p2.tile([128, 4], F32, name="sm")
            nc.vector.scalar_tensor_tensor(sm[:], eq[:], -1e30, lg, ALU.mult, ALU.add)
            m2 = p2.tile([128, 1], F32, name="m2")
            nc.vector.tensor_reduce(m2[:], sm[:], axis=AX.X, op=ALU.max)
            sel = p2.tile([128, 4], F32, name="sel")
            nc.vector.tensor_scalar(sel[:], lg, m2[:], None, ALU.is_ge)
            w4 = p2.tile([128, 4], F32, name="w4")
            nc.vector.tensor_tensor(w4[:], s[:], sel[:], ALU.mult)
            den = p2.tile([128, 1], F32, name="den")
            nc.vector.tensor_reduce(den[:], w4[:], axis=AX.X, op=ALU.add)
            dinv = p2.tile([128, 1], F32, name="dinv")
            nc.vector.reciprocal(dinv[:], den[:])
            nc.vector.tensor_scalar(rw_all[:, tt * R + 2:tt * R + 6], w4[:], dinv[:], 0.5,
                                    ALU.mult, ALU.mult)
            nc.vector.tensor_copy(msel[:, tt * NE:(tt + 1) * NE], sel[:])
            nc.vector.tensor_scalar(nsel[:, tt * NE:(tt + 1) * NE], sel[:], 1.0, -1.0,
                                    ALU.mult, ALU.add)

    # ---------------- phase 2b: silu(g) in place (one sigmoid table load) ----
    with tc.tile_pool(name="psilu", bufs=4) as psl:
        for tt in range(16):
            sg = psl.tile([128, DV], F16, name="sg")
            nc.scalar.activation(sg[:], g_sb[:, tt * DV:(tt + 1) * DV], ACTF.Sigmoid)
            nc.vector.tensor_tensor(g_sb[:, tt * DV:(tt + 1) * DV], sg[:],
                                    g_sb[:, tt * DV:(tt + 1) * DV], ALU.mult)

    # ---------------- phase 3: expert attention (combined) ----------------
    pers3 = ctx.enter_context(tc.tile_pool(name="pers3", bufs=1))
    o_acc = pers3.tile([128, 16 * DV], F16, name="o_acc")
    with tc.tile_pool(name="p3", bufs=1) as p3, \
         tc.tile_pool(name="p3ps", bufs=1, space="PSUM") as p3ps:
        for b in range(NB):
            # --- mselT: routed mask rows with keys on the free axis; each row
            #     in its own [1, TB] tile so it can be a matmul operand
            #     (base partition 0) ---
            mtp = [p3.tile([1, TB], F16, name="mtp", tag=f"mtp{i}", bufs=2)
                   for i in range(NE)]
            for kt in range(8):
                ktt = b * 8 + kt
                for i in range(NE):
                    psm = p3ps.tile([128, 128], F32, name="psm", tag="ps_misc", bufs=2)
                    nc.tensor.matmul(psm[0:1, :],
                                     msel[:, ktt * NE + i:ktt * NE + i + 1],
                                     ident16[:], start=True, stop=True)
                    nc.vector.tensor_copy(mtp[i][0:1, kt * 128:(kt + 1) * 128],
                                          psm[0:1, :])
            # --- kTm: shared set = plain kT slices; routed sets via broadcast
            #     mask tiles (ones x maskrow) + one fp16 multiply per chunk ---
            ktm = [[kT[:, dc * T + b * TB:dc * T + (b + 1) * TB] for dc in range(2)]]
            for rs in range(1, 5):
                mb = p3.tile([128, TB], F16, name="mb", tag=f"mb{rs}", bufs=1)
                for ch in range(2):
                    psb = p3ps.tile([128, 512], F32, name="psb", tag="ps_misc", bufs=2)
                    nc.tensor.matmul(psb[:], ones16[0:1, :],
                                     mtp[rs - 1][0:1, ch * 512:(ch + 1) * 512],
                                     start=True, stop=True)
                    nc.vector.tensor_copy(mb[:, ch * 512:(ch + 1) * 512], psb[:])
                pair = []
                for dc in range(2):
                    kmt = p3.tile([128, TB], F16, name="ktm", tag=f"ktm{rs}{dc}", bufs=2)
                    nc.vector.tensor_tensor(
                        kmt[:], kT[:, dc * T + b * TB:dc * T + (b + 1) * TB],
                        mb[:], ALU.mult)
                    pair.append(kmt)
                ktm.append(pair)
            # --- nspur_b[r', :] = -sum_{masked k} v[k, :]  (rank-4) ---
            psn = p3ps.tile([128, DV], F32, name="psn", tag="ps_misc", bufs=2)
            for kt in range(8):
                ktt = b * 8 + kt
                nc.tensor.matmul(psn[0:NE, :], nsel[:, ktt * NE:(ktt + 1) * NE],
                                 v_sb[:, ktt * DV:(ktt + 1) * DV],
                                 start=(kt == 0), stop=(kt == 7))
            nspur = p3.tile([NE, DV], F16, name="nspur", tag="nspur", bufs=2)
            nc.scalar.copy(nspur[:], psn[0:NE, :])
            # --- qmT for all r over this batch: [r][d2c] -> [128, TB] ---
            qmT = []
            for r in range(R):
                pair = []
                for d2c in range(2):
                    qm = p3.tile([128, TB], F16, name="qmT", tag=f"qmT{r}{d2c}", bufs=2)
                    for th in range(2):
                        psq = p3ps.tile([128, 512], F32, name="psqm", tag="ps_misc", bufs=2)
                        for dc in range(2):
                            nc.tensor.matmul(
                                psq[:],
                                wqm_sb[:, dc * 1536 + r * 256 + d2c * 128:
                                       dc * 1536 + r * 256 + d2c * 128 + 128],
                                qT[:, dc * T + b * TB + th * 512:
                                   dc * T + b * TB + th * 512 + 512],
                                start=(dc == 0), stop=(dc == 1))
                        nc.vector.tensor_copy(qm[:, th * 512:(th + 1) * 512], psq[:])
                    pair.append(qm)
                qmT.append(pair)
            for qh in range(2):
                for qt in range(4):
                    tt = b * 8 + qh * 4 + qt
                    q0 = qh * 512 + qt * 128
                    ptps = p3ps.tile([128, 1024], F32, name="ptps", tag="ptps", bufs=1)
                    csb = p3.tile([128, NE], F32, name="csb", tag="csb", bufs=2)
                    for r in range(R):
                        krs = 0 if r < 2 else r - 1
                        es_pair = []
                        dsum = []
                        for kc in range(2):
                            sps = p3ps.tile([128, 512], F32, name="sps", tag="sps", bufs=2)
                            for d2c in range(2):
                                nc.tensor.matmul(
                                    sps[:], qmT[r][d2c][:, q0:q0 + 128],
                                    ktm[krs][d2c][:, kc * 512:(kc + 1) * 512],
                                    start=(d2c == 0), stop=(d2c == 1))
                            es = p3.tile([128, 512], F16, name="es", tag="es", bufs=4)
                            dn = p3.tile([128, 1], F32, name="dn", tag="dn", bufs=4)
                            nc.scalar.activation(es[:], sps[:], ACTF.Exp, scale=SCALE,
                                                 accum_out=dn[:])
                            es_pair.append(es)
                            dsum.append(dn)
                        dtot = p3.tile([128, 1], F32, name="dtot", tag="dtot", bufs=2)
                        nc.vector.tensor_tensor(dtot[:], dsum[0][:], dsum[1][:], ALU.add)
                        dinv = p3.tile([128, 1], F32, name="adinv", tag="adinv", bufs=2)
                        nc.vector.reciprocal(dinv[:], dtot[:])
                        cmul = p3.tile([128, 1], F32, name="cmul", tag="cmul", bufs=2)
                        nc.vector.tensor_tensor(cmul[:], dinv[:],
                                                rw_all[:, tt * R + r:tt * R + r + 1],
                                                ALU.mult)
                        if r >= 2:
                            nc.vector.tensor_copy(csb[:, r - 2:r - 1], cmul[:])
                        dcd = p3.tile([128, 128], F16, name="dcd", tag="dcd", bufs=2)
                        nc.vector.tensor_scalar(dcd[:], ident16[:], cmul[:], None, ALU.mult)
                        # combine: start only on the first matmul touching each
                        # PSUM bank (start clears has_written bank-wide).
                        for kt in range(8):
                            nc.tensor.matmul(
                                ptps[:, kt * 128:(kt + 1) * 128],
                                es_pair[kt // 4][:, (kt % 4) * 128:(kt % 4) * 128 + 128],
                                dcd[:], start=(r == 0 and kt % 4 == 0),
                                stop=(r == R - 1))
                    pts = p3.tile([128, 1024], F16, name="pts", tag="pts", bufs=2)
                    nc.vector.tensor_copy(pts[:], ptps[:])
                    # CT: [4, 128] = csb^T
                    psc = p3ps.tile([128, 128], F32, name="psc", tag="ps_ct", bufs=1)
                    nc.tensor.matmul(psc[0:NE, :], csb[:], ident[:], start=True, stop=True)
                    ctb = p3.tile([NE, 128], F16, name="ctb", tag="ctb", bufs=2)
                    nc.scalar.copy(ctb[:], psc[0:NE, :])
                    # AV + spur correction
                    avp = p3ps.tile([128, DV], F32, name="avp", tag="avp", bufs=1)
                    for kt in range(8):
                        ktt = b * 8 + kt
                        nc.tensor.matmul(avp[:], pts[:, kt * 128:(kt + 1) * 128],
                                         v_sb[:, ktt * DV:(ktt + 1) * DV],
                                         start=(kt == 0), stop=False)
                    nc.tensor.matmul(avp[:], ctb[:], nspur[:], start=False, stop=True)
                    nc.vector.tensor_copy(o_acc[:, tt * DV:(tt + 1) * DV], avp[:])

    # ---------------- phase 4: gate, transpose, output projection ----------------
    with tc.tile_pool(name="p4", bufs=1) as p4, \
         tc.tile_pool(name="p4ps", bufs=1, space="PSUM") as p4ps:
        Xt = [p4.tile([128, T], F16, name=f"xt{i}", tag=f"xt{i}") for i in range(4)]
        for tt in range(16):
            xres = p4.tile([128, DV], F16, name="xres", tag="xres", bufs=3)
            nc.vector.tensor_tensor(xres[:], o_acc[:, tt * DV:(tt + 1) * DV],
                                    g_sb[:, tt * DV:(tt + 1) * DV], ALU.mult)
            for dvc in range(4):
                pst = p4ps.tile([128, 128], F16, name="pst4", tag="pst4", bufs=2)
                nc.tensor.transpose(pst[:], xres[:, dvc * 128:(dvc + 1) * 128], ident16[:])
                nc.vector.tensor_copy(Xt[dvc][:, tt * 128:(tt + 1) * 128], pst[:])
        for tt in range(16):
            for hb in range(4):
                psf = p4ps.tile([128, 512], F32, name="psf", tag="psf", bufs=2)
                for dvc in range(4):
                    nc.tensor.matmul(psf[:], Xt[dvc][:, tt * 128:(tt + 1) * 128],
                                     wo_sb[dvc][:, hb * 512:(hb + 1) * 512],
                                     start=(dvc == 0), stop=(dvc == 3))
                ost = p4.tile([128, 512], F16, name="ost", tag="ost", bufs=4)
                nc.scalar.copy(ost[:], psf[:])
                nc.sync.dma_start(out=out[tt * 128:(tt + 1) * 128, hb * 512:(hb + 1) * 512],
                                  in_=ost[:])


_PROGRAM = None


def build_program():
    global _PROGRAM
    if _PROGRAM is not None:
        return _PROGRAM
    nc = bacc.Bacc("TRN2", target_bir_lowering=False, debug=False, num_devices=8)
    names = [("wq", [HID, D], F16), ("wk", [HID, D], F16),
             ("wv", [HID, DV], F16), ("wg", [HID, DV], F16),
             ("wqm", [D, D * R], F16),
             ("hsh", [HID, T], BF16), ("hsl", [HID, T], BF16),
             ("wfh", [HID, NE], BF16), ("wfl", [HID, NE], BF16), ("wo", [DV, HID], F16)]
    io = [nc.dram_tensor(n, s, dt, kind="ExternalInput").ap() for n, s, dt in names]
    io.append(nc.dram_tensor("out", [T, HID], F16, kind="ExternalOutput").ap())
    with tile.TileContext(nc) as tc:
        from contextlib import ExitStack as ES
        with ES() as ctx:
            _body(ctx, nc, tc, io)
    nc.compile()
    _PROGRAM = nc
    return nc


def make_in_maps(hidden_states, Wq, Wk, Wv, Wq_exp, Wk_exp, Wgate, Wg, Wo):
    import ml_dtypes
    bf = ml_dtypes.bfloat16
    hs2 = np.asarray(hidden_states, np.float32).reshape(T, HID)
    hsT = np.ascontiguousarray(hs2.T)
    hsh = np.ascontiguousarray(hsT.astype(bf))
    hsl = np.ascontiguousarray((hsT.astype(np.float64) - hsh.astype(np.float64)).astype(bf))
    Wq64 = np.asarray(Wq, np.float64)
    Wg64 = np.asarray(Wgate, np.float64)
    Wqe64 = np.asarray(Wq_exp, np.float64)
    Wke64 = np.asarray(Wk_exp, np.float64)
    in_maps = []
    for c in range(8):
        wfu = Wq64[:, c * D:(c + 1) * D] @ Wg64
        wfh = wfu.astype(bf)
        wfl = (wfu - wfh.astype(np.float64)).astype(bf)
        wqm = np.empty((D, D * R), np.float16)
        for r in range(R):
            m = Wqe64[c][:, r * D:(r + 1) * D] @ Wke64[c][:, r * D:(r + 1) * D].T
            wqm[:, r * D:(r + 1) * D] = m.astype(np.float16)
        in_maps.append({
            "wq": np.asarray(Wq, np.float16)[:, c * D:(c + 1) * D].copy(),
            "wk": np.asarray(Wk, np.float16)[:, c * D:(c + 1) * D].copy(),
            "wv": np.asarray(Wv, np.float16)[:, c * DV:(c + 1) * DV].copy(),
            "wg": np.asarray(Wg, np.float16)[:, c * DV:(c + 1) * DV].copy(),
            "wqm": wqm,
            "hsh": hsh, "hsl": hsl,
            "wfh": np.ascontiguousarray(wfh), "wfl": np.ascontiguousarray(wfl),
            "wo": np.asarray(Wo, np.float16)[c * DV:(c + 1) * DV, :].copy(),
        })
    return in_maps


def kernel(hidden_states, Wq, Wk, Wv, Wq_exp, Wk_exp, Wgate, Wg, Wo):
    nc = build_program()
    in_maps = make_in_maps(hidden_states, Wq, Wk, Wv, Wq_exp, Wk_exp, Wgate, Wg, Wo)
    res = run_bass_kernel_spmd(nc, in_maps, list(range(8))).results
    out = np.zeros((T, HID), np.float32)
    for c in range(8):
        out += res[c]["out"].astype(np.float32)
    return out.reshape(2, 1024, HID).astype(np.float32)



# revision 2
# speedup vs baseline: 1.1716x; 1.1716x over previous
"""Trainium2 Bass kernel for MockMobGatedDeltaNetMoE (v9).

Sharding: head-parallel over H=8 heads, one head per NeuronCore.
Each core computes its head's full contribution; the host sums the 8
partial output projections.

v9 changes vs v8: routing (softmax top-2 over the 4 routed experts) is
computed on the host from logits = hs @ (Wq_head @ Wgate) in f64 — the
same fused-weight trick the v8 kernel ran on-device with an exact bf16
split GEMM.  The device now receives:
  - rw_sb   [128, 16*R]  f32  combine weights per token-tile
  - nsel_sb [128, 16*NE] f16  sel-1 (0 active / -1 masked) for nspur
  - mbk     [128, NE*NB*1024] f16  key masks pre-broadcast to 128 rows
This removes the 768 tiny router matmuls (LDWEIGHTS-bound, ~75us PE),
the on-device top-2 chain, the mask transpose/broadcast matmuls, and
halves the hidden-state DMA (single f16 tensor instead of bf16 hi/lo).

Pipeline (all matmuls fp16 operands, fp32 PSUM accumulation):
 - Phase 1: weights loaded once; q and k projected directly into transposed
   [d, token] layout (weight-stationary); v/g share one hs-stationary pass.
 - silu(g) precomputed in-place (one sigmoid table load, before the exp
   stream starts).
 - Score fusion: S_r = q @ (Wq_exp_r @ Wk_exp_r^T) @ k^T, M_r fused on host.
 - Key masks applied to kT with one fp16 DVE multiply per chunk;
   masked keys give exp(0)=1 = the reference denominator.
 - Scores computed transposed [query, key]; exp emits the denominator via
   accum_out.
 - Expert combine in PSUM via diagonal matmuls (transpose + scale +
   accumulate per (expert, key-tile)); one attention @ V matmul per query
   tile + rank-4 masked-v correction.
 - Phase 4: gate with precomputed silu, PE transpose, Wo projection, fp16
   partial output (host sums in fp32).
"""

import numpy as np

import concourse.bass as bass
import concourse.bacc as bacc
import concourse.tile as tile
from concourse import mybir
from concourse.bass_utils import run_bass_kernel_spmd

F32 = mybir.dt.float32
F16 = mybir.dt.float16
BF16 = mybir.dt.bfloat16
ALU = mybir.AluOpType
ACTF = mybir.ActivationFunctionType
AX = mybir.AxisListType

H, D, R, NE = 8, 256, 6, 4
HID, DV, T = 2048, 512, 2048
NB = 2
TB = T // NB
SCALE = 1.0 / 16.0


def _body(ctx, nc, tc, io):
    wq, wk, wv, wg, wqm, hst_d, rw_d, nsel_d, mbk_d, wo, out = io

    const = ctx.enter_context(tc.tile_pool(name="const", bufs=1))
    pers = ctx.enter_context(tc.tile_pool(name="pers", bufs=1))

    from concourse.masks import make_identity
    ident = const.tile([128, 128], F32, name="ident")
    make_identity(nc, ident)
    ident16 = const.tile([128, 128], F16, name="ident16")
    nc.vector.tensor_copy(ident16[:], ident[:])

    qT = pers.tile([128, 2 * T], F16, name="qT")         # [d-chunk, token]
    kT = pers.tile([128, 2 * T], F16, name="kT")         # [d-chunk, token]
    v_sb = pers.tile([128, 16 * DV], F16, name="v_sb")   # [token-tile, dv]
    g_sb = pers.tile([128, 16 * DV], F16, name="g_sb")   # [token-tile, dv]
    wqm_sb = pers.tile([128, 2 * 1536], F16, name="wqm_sb")
    rw_all = pers.tile([128, 16 * R], F32, name="rw_all")
    nsel = pers.tile([128, 16 * NE], F16, name="nsel")   # sel - 1 (0/-1)
    wo_sb = [pers.tile([128, HID], F16, name=f"wo_sb{i}") for i in range(4)]
    nc.sync.dma_start(out=rw_all[:], in_=rw_d[:, :])
    nc.sync.dma_start(out=nsel[:], in_=nsel_d[:, :])
    for dc in range(2):
        nc.sync.dma_start(out=wqm_sb[:, dc * 1536:(dc + 1) * 1536],
                          in_=wqm[dc * 128:(dc + 1) * 128, :])

    # ---------------- phase 1: projections (weights loaded once) ----------------
    with tc.tile_pool(name="p1w", bufs=1) as p1w, \
         tc.tile_pool(name="p1", bufs=1) as p1, \
         tc.tile_pool(name="p1ps", bufs=1, space="PSUM") as p1ps:
        # interleave hidden-state and weight DMAs per hid-chunk so the first
        # matmul can start after a couple of transfers.
        wq_sb, wk_sb, wv_sb, wg_sb = [], [], [], []
        hst0 = []
        for hc in range(16):
            h3 = p1.tile([128, 512], F16, name="hst", tag="hst", bufs=17)
            nc.sync.dma_start(out=h3[:], in_=hst_d[hc * 128:(hc + 1) * 128, 0:512])
            hst0.append(h3)
            for lst, src, wdt, nm in ((wq_sb, wq, 256, "wqsb"), (wk_sb, wk, 256, "wksb"),
                                      (wv_sb, wv, 512, "wvsb"), (wg_sb, wg, 512, "wgsb")):
                w1 = p1w.tile([128, wdt], F16, name=nm, tag=f"{nm}{hc}")
                nc.sync.dma_start(out=w1[:], in_=src[hc * 128:(hc + 1) * 128, :])
                lst.append(w1)
        for i in range(4):
            nc.sync.dma_start(out=wo_sb[i][:], in_=wo[i * 128:(i + 1) * 128, :])
        for tb in range(4):  # token blocks of 512
            t0 = tb * 512
            if tb == 0:
                hst = hst0
            else:
                hst = []
                for hc in range(16):
                    h3 = p1.tile([128, 512], F16, name="hst", tag="hst", bufs=17)
                    nc.sync.dma_start(out=h3[:], in_=hst_d[hc * 128:(hc + 1) * 128, t0:t0 + 512])
                    hst.append(h3)
            # q/k projections -> transposed [d-chunk, token] (weight-stationary)
            for wsb, dstT in ((wq_sb, qT), (wk_sb, kT)):
                for f in range(2):
                    ps = p1ps.tile([128, 512], F32, name="psq", tag="psq", bufs=2)
                    for hc in range(16):
                        nc.tensor.matmul(ps[:], wsb[hc][:, f * 128:(f + 1) * 128],
                                         hst[hc][:], start=(hc == 0), stop=(hc == 15))
                    nc.scalar.copy(dstT[:, f * T + t0:f * T + t0 + 512], ps[:])
            # v/g: one shared-stationary pass per token tile
            for tt4 in range(4):
                tt = tb * 4 + tt4
                ps = p1ps.tile([128, 1024], F32, name="psvg", tag="psvg", bufs=2)
                for hc in range(16):
                    st_ap = hst[hc][:, tt4 * 128:(tt4 + 1) * 128]
                    nc.tensor.matmul(ps[:, 0:512], st_ap, wv_sb[hc][:],
                                     start=(hc == 0), stop=(hc == 15))
                    nc.tensor.matmul(ps[:, 512:1024], st_ap, wg_sb[hc][:],
                                     start=(hc == 0), stop=(hc == 15))
                nc.scalar.copy(v_sb[:, tt * DV:(tt + 1) * DV], ps[:, 0:512])
                nc.vector.tensor_copy(g_sb[:, tt * DV:(tt + 1) * DV], ps[:, 512:1024])

    # ---------------- phase 2b: silu(g) in place (one sigmoid table load) ----
    with tc.tile_pool(name="psilu", bufs=4) as psl:
        for tt in range(16):
            sg = psl.tile([128, DV], F16, name="sg")
            nc.scalar.activation(sg[:], g_sb[:, tt * DV:(tt + 1) * DV], ACTF.Sigmoid)
            nc.vector.tensor_tensor(g_sb[:, tt * DV:(tt + 1) * DV], sg[:],
                                    g_sb[:, tt * DV:(tt + 1) * DV], ALU.mult)

    # ---------------- phase 3: expert attention (combined) ----------------
    pers3 = ctx.enter_context(tc.tile_pool(name="pers3", bufs=1))
    o_acc = pers3.tile([128, 16 * DV], F16, name="o_acc")
    with tc.tile_pool(name="p3", bufs=1) as p3, \
         tc.tile_pool(name="p3ps", bufs=1, space="PSUM") as p3ps:
        for b in range(NB):
            # --- kTm: shared set = plain kT slices; routed sets via
            #     host-shipped broadcast mask tiles + one fp16 multiply ---
            ktm = [[kT[:, dc * T + b * TB:dc * T + (b + 1) * TB] for dc in range(2)]]
            for rs in range(1, 5):
                mb = p3.tile([128, TB], F16, name="mb", tag=f"mb{rs}", bufs=1)
                nc.sync.dma_start(
                    out=mb[:],
                    in_=mbk_d[:, ((rs - 1) * NB + b) * TB:((rs - 1) * NB + b + 1) * TB])
                pair = []
                for dc in range(2):
                    kmt = p3.tile([128, TB], F16, name="ktm", tag=f"ktm{rs}{dc}", bufs=2)
                    nc.vector.tensor_tensor(
                        kmt[:], kT[:, dc * T + b * TB:dc * T + (b + 1) * TB],
                        mb[:], ALU.mult)
                    pair.append(kmt)
                ktm.append(pair)
            # --- nspur_b[r', :] = -sum_{masked k} v[k, :]  (rank-4) ---
            psn = p3ps.tile([128, DV], F32, name="psn", tag="ps_misc", bufs=2)
            for kt in range(8):
                ktt = b * 8 + kt
                nc.tensor.matmul(psn[0:NE, :], nsel[:, ktt * NE:(ktt + 1) * NE],
                                 v_sb[:, ktt * DV:(ktt + 1) * DV],
                                 start=(kt == 0), stop=(kt == 7))
            nspur = p3.tile([NE, DV], F16, name="nspur", tag="nspur", bufs=2)
            nc.scalar.copy(nspur[:], psn[0:NE, :])
            # --- qmT for all r over this batch: [r][d2c] -> [128, TB] ---
            qmT = []
            for r in range(R):
                pair = []
                for d2c in range(2):
                    qm = p3.tile([128, TB], F16, name="qmT", tag=f"qmT{r}{d2c}", bufs=2)
                    for th in range(2):
                        psq = p3ps.tile([128, 512], F32, name="psqm", tag="ps_misc", bufs=2)
                        for dc in range(2):
                            nc.tensor.matmul(
                                psq[:],
                                wqm_sb[:, dc * 1536 + r * 256 + d2c * 128:
                                       dc * 1536 + r * 256 + d2c * 128 + 128],
                                qT[:, dc * T + b * TB + th * 512:
                                   dc * T + b * TB + th * 512 + 512],
                                start=(dc == 0), stop=(dc == 1))
                        nc.vector.tensor_copy(qm[:, th * 512:(th + 1) * 512], psq[:])
                    pair.append(qm)
                qmT.append(pair)
            for qh in range(2):
                for qt in range(4):
                    tt = b * 8 + qh * 4 + qt
                    q0 = qh * 512 + qt * 128
                    ptps = p3ps.tile([128, 1024], F32, name="ptps", tag="ptps", bufs=1)
                    csb = p3.tile([128, NE], F32, name="csb", tag="csb", bufs=2)
                    for r in range(R):
                        krs = 0 if r < 2 else r - 1
                        es_pair = []
                        dsum = []
                        for kc in range(2):
                            sps = p3ps.tile([128, 512], F32, name="sps", tag="sps", bufs=2)
                            for d2c in range(2):
                                nc.tensor.matmul(
                                    sps[:], qmT[r][d2c][:, q0:q0 + 128],
                                    ktm[krs][d2c][:, kc * 512:(kc + 1) * 512],
                                    start=(d2c == 0), stop=(d2c == 1))
                            es = p3.tile([128, 512], F16, name="es", tag="es", bufs=4)
                            dn = p3.tile([128, 1], F32, name="dn", tag="dn", bufs=4)
                            nc.scalar.activation(es[:], sps[:], ACTF.Exp, scale=SCALE,
                                                 accum_out=dn[:])
                            es_pair.append(es)
                            dsum.append(dn)
                        dtot = p3.tile([128, 1], F32, name="dtot", tag="dtot", bufs=2)
                        nc.vector.tensor_tensor(dtot[:], dsum[0][:], dsum[1][:], ALU.add)
                        dinv = p3.tile([128, 1], F32, name="adinv", tag="adinv", bufs=2)
                        nc.vector.reciprocal(dinv[:], dtot[:])
                        cmul = p3.tile([128, 1], F32, name="cmul", tag="cmul", bufs=2)
                        nc.vector.tensor_tensor(cmul[:], dinv[:],
                                                rw_all[:, tt * R + r:tt * R + r + 1],
                                                ALU.mult)
                        if r >= 2:
                            nc.vector.tensor_copy(csb[:, r - 2:r - 1], cmul[:])
                        dcd = p3.tile([128, 128], F16, name="dcd", tag="dcd", bufs=2)
                        nc.vector.tensor_scalar(dcd[:], ident16[:], cmul[:], None, ALU.mult)
                        # combine: start only on the first matmul touching each
                        # PSUM bank (start clears has_written bank-wide).
                        for kt in range(8):
                            nc.tensor.matmul(
                                ptps[:, kt * 128:(kt + 1) * 128],
                                es_pair[kt // 4][:, (kt % 4) * 128:(kt % 4) * 128 + 128],
                                dcd[:], start=(r == 0 and kt % 4 == 0),
                                stop=(r == R - 1))
                    pts = p3.tile([128, 1024], F16, name="pts", tag="pts", bufs=2)
                    nc.vector.tensor_copy(pts[:], ptps[:])
                    # CT: [4, 128] = csb^T
                    psc = p3ps.tile([128, 128], F32, name="psc", tag="ps_ct", bufs=1)
                    nc.tensor.matmul(psc[0:NE, :], csb[:], ident[:], start=True, stop=True)
                    ctb = p3.tile([NE, 128], F16, name="ctb", tag="ctb", bufs=2)
                    nc.scalar.copy(ctb[:], psc[0:NE, :])
                    # AV + spur correction
                    avp = p3ps.tile([128, DV], F32, name="avp", tag="avp", bufs=1)
                    for kt in range(8):
                        ktt = b * 8 + kt
                        nc.tensor.matmul(avp[:], pts[:, kt * 128:(kt + 1) * 128],
                                         v_sb[:, ktt * DV:(ktt + 1) * DV],
                                         start=(kt == 0), stop=False)
                    nc.tensor.matmul(avp[:], ctb[:], nspur[:], start=False, stop=True)
                    nc.vector.tensor_copy(o_acc[:, tt * DV:(tt + 1) * DV], avp[:])

    # ---------------- phase 4: gate, transpose, output projection ----------------
    with tc.tile_pool(name="p4", bufs=1) as p4, \
         tc.tile_pool(name="p4ps", bufs=1, space="PSUM") as p4ps:
        Xt = [p4.tile([128, T], F16, name=f"xt{i}", tag=f"xt{i}") for i in range(4)]
        for tt in range(16):
            xres = p4.tile([128, DV], F16, name="xres", tag="xres", bufs=3)
            nc.vector.tensor_tensor(xres[:], o_acc[:, tt * DV:(tt + 1) * DV],
                                    g_sb[:, tt * DV:(tt + 1) * DV], ALU.mult)
            for dvc in range(4):
                pst = p4ps.tile([128, 128], F16, name="pst4", tag="pst4", bufs=2)
                nc.tensor.transpose(pst[:], xres[:, dvc * 128:(dvc + 1) * 128], ident16[:])
                nc.vector.tensor_copy(Xt[dvc][:, tt * 128:(tt + 1) * 128], pst[:])
        for tt in range(16):
            for hb in range(4):
                psf = p4ps.tile([128, 512], F32, name="psf", tag="psf", bufs=2)
                for dvc in range(4):
                    nc.tensor.matmul(psf[:], Xt[dvc][:, tt * 128:(tt + 1) * 128],
                                     wo_sb[dvc][:, hb * 512:(hb + 1) * 512],
                                     start=(dvc == 0), stop=(dvc == 3))
                ost = p4.tile([128, 512], F16, name="ost", tag="ost", bufs=4)
                nc.scalar.copy(ost[:], psf[:])
                nc.sync.dma_start(out=out[tt * 128:(tt + 1) * 128, hb * 512:(hb + 1) * 512],
                                  in_=ost[:])


_PROGRAM = None


def build_program():
    global _PROGRAM
    if _PROGRAM is not None:
        return _PROGRAM
    nc = bacc.Bacc("TRN2", target_bir_lowering=False, debug=False, num_devices=8)
    names = [("wq", [HID, D], F16), ("wk", [HID, D], F16),
             ("wv", [HID, DV], F16), ("wg", [HID, DV], F16),
             ("wqm", [D, D * R], F16),
             ("hst", [HID, T], F16),
             ("rw", [128, 16 * R], F32), ("nsel", [128, 16 * NE], F16),
             ("mbk", [128, NE * NB * TB], F16), ("wo", [DV, HID], F16)]
    io = [nc.dram_tensor(n, s, dt, kind="ExternalInput").ap() for n, s, dt in names]
    io.append(nc.dram_tensor("out", [T, HID], F16, kind="ExternalOutput").ap())
    with tile.TileContext(nc) as tc:
        from contextlib import ExitStack as ES
        with ES() as ctx:
            _body(ctx, nc, tc, io)
    nc.compile()
    _PROGRAM = nc
    return nc


def _host_routing(hs64, Wq64, Wgate64):
    """Per-head routing on host, matching the reference bit-for-bit in f64.

    Returns rw [T, R] f32, msel [T, NE] f16 (1 active / 0 masked)."""
    S, K = 2, 2
    T_, _ = hs64.shape
    rw = np.zeros((T_, R), np.float64)
    rw[:, :S] = 0.25
    logits = hs64 @ (Wq64 @ Wgate64)          # [T, R-S]
    sc = np.exp(logits - logits.max(axis=-1, keepdims=True))
    sc /= sc.sum(axis=-1, keepdims=True)
    idx = np.argsort(-sc, axis=-1, kind="stable")[:, :K]   # top-2, ties -> low idx
    w = np.take_along_axis(sc, idx, axis=-1)
    w /= w.sum(axis=-1, keepdims=True)
    np.put_along_axis(rw[:, S:], idx, w * 0.5, axis=-1)
    msel = np.zeros((T_, NE), np.float64)
    np.put_along_axis(msel, idx, 1.0, axis=-1)
    return rw.astype(np.float32), msel.astype(np.float16)


def make_in_maps(hidden_states, Wq, Wk, Wv, Wq_exp, Wk_exp, Wgate, Wg, Wo):
    hs2 = np.asarray(hidden_states, np.float32).reshape(T, HID)
    hsT = np.ascontiguousarray(hs2.T.astype(np.float16))
    hs64 = hs2.astype(np.float64)
    Wq64 = np.asarray(Wq, np.float64)
    Wg64 = np.asarray(Wgate, np.float64)
    Wqe64 = np.asarray(Wq_exp, np.float64)
    Wke64 = np.asarray(Wk_exp, np.float64)
    in_maps = []
    for c in range(8):
        rw, msel = _host_routing(hs64, Wq64[:, c * D:(c + 1) * D], Wg64)
        # rw_sb [128, 16*R]: token-tile-major combine weights
        rw_sb = np.ascontiguousarray(
            rw.reshape(16, 128, R).transpose(1, 0, 2).reshape(128, 16 * R))
        nsel_sb = np.ascontiguousarray(
            (msel - 1).reshape(16, 128, NE).transpose(1, 0, 2).reshape(128, 16 * NE)
        ).astype(np.float16)
        # mbk [128, NE*NB*TB]: key mask broadcast to 128 partitions
        mbk = np.empty((128, NE * NB * TB), np.float16)
        for i in range(NE):
            for b in range(NB):
                mbk[:, (i * NB + b) * TB:(i * NB + b + 1) * TB] = \
                    msel[b * TB:(b + 1) * TB, i][None, :]
        wqm = np.empty((D, D * R), np.float16)
        for r in range(R):
            m = Wqe64[c][:, r * D:(r + 1) * D] @ Wke64[c][:, r * D:(r + 1) * D].T
            wqm[:, r * D:(r + 1) * D] = m.astype(np.float16)
        in_maps.append({
            "wq": np.asarray(Wq, np.float16)[:, c * D:(c + 1) * D].copy(),
            "wk": np.asarray(Wk, np.float16)[:, c * D:(c + 1) * D].copy(),
            "wv": np.asarray(Wv, np.float16)[:, c * DV:(c + 1) * DV].copy(),
            "wg": np.asarray(Wg, np.float16)[:, c * DV:(c + 1) * DV].copy(),
            "wqm": wqm,
            "hst": hsT,
            "rw": rw_sb, "nsel": nsel_sb, "mbk": mbk,
            "wo": np.asarray(Wo, np.float16)[c * DV:(c + 1) * DV, :].copy(),
        })
    return in_maps


def kernel(hidden_states, Wq, Wk, Wv, Wq_exp, Wk_exp, Wgate, Wg, Wo):
    nc = build_program()
    in_maps = make_in_maps(hidden_states, Wq, Wk, Wv, Wq_exp, Wk_exp, Wgate, Wg, Wo)
    res = run_bass_kernel_spmd(nc, in_maps, list(range(8))).results
    out = np.zeros((T, HID), np.float32)
    for c in range(8):
        out += res[c]["out"].astype(np.float32)
    return out.reshape(2, 1024, HID).astype(np.float32)


# revision 3
# speedup vs baseline: 1.2124x; 1.0349x over previous
"""Trainium2 Bass kernel for MockMobGatedDeltaNetMoE (v10).

Sharding: head-parallel over H=8 heads, one head per NeuronCore.
Each core computes its head's full contribution; the host sums the 8
partial output projections.

v9: routing (softmax top-2) computed on host from
logits = hs @ (Wq_head @ Wgate) in f64; device receives rw/nsel/mask
tensors.  Removes the 768 LDWEIGHTS-bound router matmuls, the on-device
top-2 chain, the mask transpose matmuls, and halves hidden-state DMA.

v10:
 - phase-1 DMA stream reordered (hst+wq pairs first, then wk/wv/wg) so
   the first q matmul chain starts after ~2 transfers instead of ~29us.
 - single merged exp per (query-tile, expert): scores land in one
   [128,1024] 2-bank PSUM tile, one activation + one accum_out gives the
   full denominator (halves ACT instruction count, kills the dtot adds).
 - qm PSUM merged to [128,1024] per (r,d2c): one copy instead of two.
 - per-qt cmul collected in a [128,6] tile; CT transpose reads its
   routed columns directly (no csb copies).
 - phase-4 output staging copies split between scalar and vector engines.
"""

import numpy as np

import concourse.bass as bass
import concourse.bacc as bacc
import concourse.tile as tile
from concourse import mybir
from concourse.bass_utils import run_bass_kernel_spmd

F32 = mybir.dt.float32
F16 = mybir.dt.float16
BF16 = mybir.dt.bfloat16
ALU = mybir.AluOpType
ACTF = mybir.ActivationFunctionType
AX = mybir.AxisListType

H, D, R, NE = 8, 256, 6, 4
HID, DV, T = 2048, 512, 2048
NB = 2
TB = T // NB
SCALE = 1.0 / 16.0


def _body(ctx, nc, tc, io):
    wq, wk, wv, wg, wqm, hst_d, rw_d, nsel_d, mbk_d, wo, out = io

    const = ctx.enter_context(tc.tile_pool(name="const", bufs=1))
    pers = ctx.enter_context(tc.tile_pool(name="pers", bufs=1))

    from concourse.masks import make_identity
    ident = const.tile([128, 128], F32, name="ident")
    make_identity(nc, ident)
    ident16 = const.tile([128, 128], F16, name="ident16")
    nc.vector.tensor_copy(ident16[:], ident[:])

    qT = pers.tile([128, 2 * T], F16, name="qT")         # [d-chunk, token]
    kT = pers.tile([128, 2 * T], F16, name="kT")         # [d-chunk, token]
    v_sb = pers.tile([128, 16 * DV], F16, name="v_sb")   # [token-tile, dv]
    g_sb = pers.tile([128, 16 * DV], F16, name="g_sb")   # [token-tile, dv]
    wqm_sb = pers.tile([128, 2 * 1536], F16, name="wqm_sb")
    rw_all = pers.tile([128, 16 * R], F32, name="rw_all")
    nsel = pers.tile([128, 16 * NE], F16, name="nsel")   # sel - 1 (0/-1)
    wo_sb = [pers.tile([128, HID], F16, name=f"wo_sb{i}") for i in range(4)]

    # ---------------- phase 1: projections (weights loaded once) ----------------
    with tc.tile_pool(name="p1w", bufs=1) as p1w, \
         tc.tile_pool(name="p1", bufs=1) as p1, \
         tc.tile_pool(name="p1ps", bufs=1, space="PSUM") as p1ps:
        # DMA order: (hst, wq) pairs first so the q chain starts immediately,
        # then wk, wv, wg streams; small/late tensors after.
        wq_sb, wk_sb, wv_sb, wg_sb = [], [], [], []
        hst0 = []
        for hc in range(16):
            h3 = p1.tile([128, 512], F16, name="hst", tag="hst", bufs=17)
            nc.sync.dma_start(out=h3[:], in_=hst_d[hc * 128:(hc + 1) * 128, 0:512])
            hst0.append(h3)
            w1 = p1w.tile([128, 256], F16, name="wqsb", tag=f"wqsb{hc}")
            nc.sync.dma_start(out=w1[:], in_=wq[hc * 128:(hc + 1) * 128, :])
            wq_sb.append(w1)
        for lst, src, wdt, nm in ((wk_sb, wk, 256, "wksb"),
                                  (wv_sb, wv, 512, "wvsb"), (wg_sb, wg, 512, "wgsb")):
            for hc in range(16):
                w1 = p1w.tile([128, wdt], F16, name=nm, tag=f"{nm}{hc}")
                nc.sync.dma_start(out=w1[:], in_=src[hc * 128:(hc + 1) * 128, :])
                lst.append(w1)
        nc.sync.dma_start(out=rw_all[:], in_=rw_d[:, :])
        nc.sync.dma_start(out=nsel[:], in_=nsel_d[:, :])
        for dc in range(2):
            nc.sync.dma_start(out=wqm_sb[:, dc * 1536:(dc + 1) * 1536],
                              in_=wqm[dc * 128:(dc + 1) * 128, :])
        for i in range(4):
            nc.sync.dma_start(out=wo_sb[i][:], in_=wo[i * 128:(i + 1) * 128, :])
        for tb in range(4):  # token blocks of 512
            t0 = tb * 512
            if tb == 0:
                hst = hst0
            else:
                hst = []
                for hc in range(16):
                    h3 = p1.tile([128, 512], F16, name="hst", tag="hst", bufs=17)
                    nc.sync.dma_start(out=h3[:], in_=hst_d[hc * 128:(hc + 1) * 128, t0:t0 + 512])
                    hst.append(h3)
            # q/k projections -> transposed [d-chunk, token] (weight-stationary)
            for wsb, dstT in ((wq_sb, qT), (wk_sb, kT)):
                for f in range(2):
                    ps = p1ps.tile([128, 512], F32, name="psq", tag="psq", bufs=2)
                    for hc in range(16):
                        nc.tensor.matmul(ps[:], wsb[hc][:, f * 128:(f + 1) * 128],
                                         hst[hc][:], start=(hc == 0), stop=(hc == 15))
                    nc.scalar.copy(dstT[:, f * T + t0:f * T + t0 + 512], ps[:])
            # v/g: one shared-stationary pass per token tile
            for tt4 in range(4):
                tt = tb * 4 + tt4
                ps = p1ps.tile([128, 1024], F32, name="psvg", tag="psvg", bufs=2)
                for hc in range(16):
                    st_ap = hst[hc][:, tt4 * 128:(tt4 + 1) * 128]
                    nc.tensor.matmul(ps[:, 0:512], st_ap, wv_sb[hc][:],
                                     start=(hc == 0), stop=(hc == 15))
                    nc.tensor.matmul(ps[:, 512:1024], st_ap, wg_sb[hc][:],
                                     start=(hc == 0), stop=(hc == 15))
                nc.scalar.copy(v_sb[:, tt * DV:(tt + 1) * DV], ps[:, 0:512])
                nc.vector.tensor_copy(g_sb[:, tt * DV:(tt + 1) * DV], ps[:, 512:1024])

    # ---------------- phase 2b: silu(g) in place (one sigmoid table load) ----
    with tc.tile_pool(name="psilu", bufs=4) as psl:
        for tt in range(16):
            sg = psl.tile([128, DV], F16, name="sg")
            nc.scalar.activation(sg[:], g_sb[:, tt * DV:(tt + 1) * DV], ACTF.Sigmoid)
            nc.vector.tensor_tensor(g_sb[:, tt * DV:(tt + 1) * DV], sg[:],
                                    g_sb[:, tt * DV:(tt + 1) * DV], ALU.mult)

    # ---------------- phase 3: expert attention (combined) ----------------
    pers3 = ctx.enter_context(tc.tile_pool(name="pers3", bufs=1))
    o_acc = pers3.tile([128, 16 * DV], F16, name="o_acc")
    with tc.tile_pool(name="p3", bufs=1) as p3, \
         tc.tile_pool(name="p3ps", bufs=1, space="PSUM") as p3ps:
        for b in range(NB):
            # --- kTm: shared set = plain kT slices; routed sets via
            #     host-shipped broadcast mask tiles + one fp16 multiply ---
            ktm = [[kT[:, dc * T + b * TB:dc * T + (b + 1) * TB] for dc in range(2)]]
            for rs in range(1, 5):
                mb = p3.tile([128, TB], F16, name="mb", tag=f"mb{rs}", bufs=2)
                nc.sync.dma_start(
                    out=mb[:],
                    in_=mbk_d[:, ((rs - 1) * NB + b) * TB:((rs - 1) * NB + b + 1) * TB])
                pair = []
                for dc in range(2):
                    kmt = p3.tile([128, TB], F16, name="ktm", tag=f"ktm{rs}{dc}", bufs=2)
                    nc.vector.tensor_tensor(
                        kmt[:], kT[:, dc * T + b * TB:dc * T + (b + 1) * TB],
                        mb[:], ALU.mult)
                    pair.append(kmt)
                ktm.append(pair)
            # --- nspur_b[r', :] = -sum_{masked k} v[k, :]  (rank-4) ---
            psn = p3ps.tile([128, 1024], F32, name="psn", tag="sps", bufs=2)
            for kt in range(8):
                ktt = b * 8 + kt
                nc.tensor.matmul(psn[0:NE, 0:DV], nsel[:, ktt * NE:(ktt + 1) * NE],
                                 v_sb[:, ktt * DV:(ktt + 1) * DV],
                                 start=(kt == 0), stop=(kt == 7))
            nspur = p3.tile([NE, DV], F16, name="nspur", tag="nspur", bufs=2)
            nc.scalar.copy(nspur[:], psn[0:NE, 0:DV])
            # --- qmT for all r over this batch: [r][d2c] -> [128, TB] ---
            qmT = []
            for r in range(R):
                pair = []
                for d2c in range(2):
                    qm = p3.tile([128, TB], F16, name="qmT", tag=f"qmT{r}{d2c}", bufs=2)
                    psq = p3ps.tile([128, 1024], F32, name="psqm", tag="sps", bufs=2)
                    for th in range(2):
                        for dc in range(2):
                            nc.tensor.matmul(
                                psq[:, th * 512:(th + 1) * 512],
                                wqm_sb[:, dc * 1536 + r * 256 + d2c * 128:
                                       dc * 1536 + r * 256 + d2c * 128 + 128],
                                qT[:, dc * T + b * TB + th * 512:
                                   dc * T + b * TB + th * 512 + 512],
                                start=(dc == 0), stop=(dc == 1))
                    nc.vector.tensor_copy(qm[:], psq[:])
                    pair.append(qm)
                qmT.append(pair)
            for qh in range(2):
                for qt in range(4):
                    tt = b * 8 + qh * 4 + qt
                    q0 = qh * 512 + qt * 128
                    ptps = p3ps.tile([128, 1024], F32, name="ptps", tag="ptps", bufs=1)
                    cmul_all = p3.tile([128, R], F32, name="cmul_all", tag="cmula", bufs=2)
                    for r in range(R):
                        krs = 0 if r < 2 else r - 1
                        sps = p3ps.tile([128, 1024], F32, name="sps", tag="sps", bufs=2)
                        for kc in range(2):
                            for d2c in range(2):
                                nc.tensor.matmul(
                                    sps[:, kc * 512:(kc + 1) * 512],
                                    qmT[r][d2c][:, q0:q0 + 128],
                                    ktm[krs][d2c][:, kc * 512:(kc + 1) * 512],
                                    start=(d2c == 0), stop=(d2c == 1))
                        es = p3.tile([128, 1024], F16, name="es", tag="es", bufs=3)
                        dn = p3.tile([128, 1], F32, name="dn", tag="dn", bufs=4)
                        nc.scalar.activation(es[:], sps[:], ACTF.Exp, scale=SCALE,
                                             accum_out=dn[:])
                        dinv = p3.tile([128, 1], F32, name="adinv", tag="adinv", bufs=2)
                        nc.vector.reciprocal(dinv[:], dn[:])
                        nc.vector.tensor_tensor(cmul_all[:, r:r + 1], dinv[:],
                                                rw_all[:, tt * R + r:tt * R + r + 1],
                                                ALU.mult)
                        dcd = p3.tile([128, 128], F16, name="dcd", tag="dcd", bufs=2)
                        nc.vector.tensor_scalar(dcd[:], ident16[:], cmul_all[:, r:r + 1],
                                                None, ALU.mult)
                        # combine: start only on the first matmul touching each
                        # PSUM bank (start clears has_written bank-wide).
                        for kt in range(8):
                            nc.tensor.matmul(
                                ptps[:, kt * 128:(kt + 1) * 128],
                                es[:, kt * 128:(kt + 1) * 128],
                                dcd[:], start=(r == 0 and kt % 4 == 0),
                                stop=(r == R - 1))
                    pts = p3.tile([128, 1024], F16, name="pts", tag="pts", bufs=2)
                    nc.vector.tensor_copy(pts[:], ptps[:])
                    # CT: [4, 128] = cmul_all[:, 2:6]^T
                    psc = p3ps.tile([128, 128], F32, name="psc", tag="ps_ct", bufs=1)
                    nc.tensor.matmul(psc[0:NE, :], cmul_all[:, 2:R], ident[:],
                                     start=True, stop=True)
                    ctb = p3.tile([NE, 128], F16, name="ctb", tag="ctb", bufs=2)
                    nc.scalar.copy(ctb[:], psc[0:NE, :])
                    # AV + spur correction
                    avp = p3ps.tile([128, DV], F32, name="avp", tag="avp", bufs=1)
                    for kt in range(8):
                        ktt = b * 8 + kt
                        nc.tensor.matmul(avp[:], pts[:, kt * 128:(kt + 1) * 128],
                                         v_sb[:, ktt * DV:(ktt + 1) * DV],
                                         start=(kt == 0), stop=False)
                    nc.tensor.matmul(avp[:], ctb[:], nspur[:], start=False, stop=True)
                    nc.vector.tensor_copy(o_acc[:, tt * DV:(tt + 1) * DV], avp[:])

    # ---------------- phase 4: gate, transpose, output projection ----------------
    with tc.tile_pool(name="p4", bufs=1) as p4, \
         tc.tile_pool(name="p4ps", bufs=1, space="PSUM") as p4ps:
        Xt = [p4.tile([128, T], F16, name=f"xt{i}", tag=f"xt{i}") for i in range(4)]
        for tt in range(16):
            xres = p4.tile([128, DV], F16, name="xres", tag="xres", bufs=3)
            nc.vector.tensor_tensor(xres[:], o_acc[:, tt * DV:(tt + 1) * DV],
                                    g_sb[:, tt * DV:(tt + 1) * DV], ALU.mult)
            for dvc in range(4):
                pst = p4ps.tile([128, 128], F16, name="pst4", tag="pst4", bufs=2)
                nc.tensor.transpose(pst[:], xres[:, dvc * 128:(dvc + 1) * 128], ident16[:])
                nc.vector.tensor_copy(Xt[dvc][:, tt * 128:(tt + 1) * 128], pst[:])
        for tt in range(16):
            for hb in range(4):
                psf = p4ps.tile([128, 512], F32, name="psf", tag="psf", bufs=2)
                for dvc in range(4):
                    nc.tensor.matmul(psf[:], Xt[dvc][:, tt * 128:(tt + 1) * 128],
                                     wo_sb[dvc][:, hb * 512:(hb + 1) * 512],
                                     start=(dvc == 0), stop=(dvc == 3))
                ost = p4.tile([128, 512], F16, name="ost", tag="ost", bufs=4)
                if (tt * 4 + hb) % 2 == 0:
                    nc.scalar.copy(ost[:], psf[:])
                else:
                    nc.vector.tensor_copy(ost[:], psf[:])
                nc.sync.dma_start(out=out[tt * 128:(tt + 1) * 128, hb * 512:(hb + 1) * 512],
                                  in_=ost[:])


_PROGRAM = None


def build_program():
    global _PROGRAM
    if _PROGRAM is not None:
        return _PROGRAM
    nc = bacc.Bacc("TRN2", target_bir_lowering=False, debug=False, num_devices=8)
    names = [("wq", [HID, D], F16), ("wk", [HID, D], F16),
             ("wv", [HID, DV], F16), ("wg", [HID, DV], F16),
             ("wqm", [D, D * R], F16),
             ("hst", [HID, T], F16),
             ("rw", [128, 16 * R], F32), ("nsel", [128, 16 * NE], F16),
             ("mbk", [128, NE * NB * TB], F16), ("wo", [DV, HID], F16)]
    io = [nc.dram_tensor(n, s, dt, kind="ExternalInput").ap() for n, s, dt in names]
    io.append(nc.dram_tensor("out", [T, HID], F16, kind="ExternalOutput").ap())
    with tile.TileContext(nc) as tc:
        from contextlib import ExitStack as ES
        with ES() as ctx:
            _body(ctx, nc, tc, io)
    nc.compile()
    _PROGRAM = nc
    return nc


def _host_routing(hs64, Wq64, Wgate64):
    """Per-head routing on host, matching the reference bit-for-bit in f64.

    Returns rw [T, R] f32, msel [T, NE] f16 (1 active / 0 masked)."""
    S, K = 2, 2
    T_, _ = hs64.shape
    rw = np.zeros((T_, R), np.float64)
    rw[:, :S] = 0.25
    logits = hs64 @ (Wq64 @ Wgate64)          # [T, R-S]
    sc = np.exp(logits - logits.max(axis=-1, keepdims=True))
    sc /= sc.sum(axis=-1, keepdims=True)
    idx = np.argsort(-sc, axis=-1, kind="stable")[:, :K]   # top-2, ties -> low idx
    w = np.take_along_axis(sc, idx, axis=-1)
    w /= w.sum(axis=-1, keepdims=True)
    np.put_along_axis(rw[:, S:], idx, w * 0.5, axis=-1)
    msel = np.zeros((T_, NE), np.float64)
    np.put_along_axis(msel, idx, 1.0, axis=-1)
    return rw.astype(np.float32), msel.astype(np.float16)


def make_in_maps(hidden_states, Wq, Wk, Wv, Wq_exp, Wk_exp, Wgate, Wg, Wo):
    hs2 = np.asarray(hidden_states, np.float32).reshape(T, HID)
    hsT = np.ascontiguousarray(hs2.T.astype(np.float16))
    hs64 = hs2.astype(np.float64)
    Wq64 = np.asarray(Wq, np.float64)
    Wg64 = np.asarray(Wgate, np.float64)
    Wqe64 = np.asarray(Wq_exp, np.float64)
    Wke64 = np.asarray(Wk_exp, np.float64)
    in_maps = []
    for c in range(8):
        rw, msel = _host_routing(hs64, Wq64[:, c * D:(c + 1) * D], Wg64)
        # rw_sb [128, 16*R]: token-tile-major combine weights
        rw_sb = np.ascontiguousarray(
            rw.reshape(16, 128, R).transpose(1, 0, 2).reshape(128, 16 * R))
        nsel_sb = np.ascontiguousarray(
            (msel - 1).reshape(16, 128, NE).transpose(1, 0, 2).reshape(128, 16 * NE)
        ).astype(np.float16)
        # mbk [128, NE*NB*TB]: key mask broadcast to 128 partitions
        mbk = np.empty((128, NE * NB * TB), np.float16)
        for i in range(NE):
            for b in range(NB):
                mbk[:, (i * NB + b) * TB:(i * NB + b + 1) * TB] = \
                    msel[b * TB:(b + 1) * TB, i][None, :]
        wqm = np.empty((D, D * R), np.float16)
        for r in range(R):
            m = Wqe64[c][:, r * D:(r + 1) * D] @ Wke64[c][:, r * D:(r + 1) * D].T
            wqm[:, r * D:(r + 1) * D] = m.astype(np.float16)
        in_maps.append({
            "wq": np.asarray(Wq, np.float16)[:, c * D:(c + 1) * D].copy(),
            "wk": np.asarray(Wk, np.float16)[:, c * D:(c + 1) * D].copy(),
            "wv": np.asarray(Wv, np.float16)[:, c * DV:(c + 1) * DV].copy(),
            "wg": np.asarray(Wg, np.float16)[:, c * DV:(c + 1) * DV].copy(),
            "wqm": wqm,
            "hst": hsT,
            "rw": rw_sb, "nsel": nsel_sb, "mbk": mbk,
            "wo": np.asarray(Wo, np.float16)[c * DV:(c + 1) * DV, :].copy(),
        })
    return in_maps


def kernel(hidden_states, Wq, Wk, Wv, Wq_exp, Wk_exp, Wgate, Wg, Wo):
    nc = build_program()
    in_maps = make_in_maps(hidden_states, Wq, Wk, Wv, Wq_exp, Wk_exp, Wgate, Wg, Wo)
    res = run_bass_kernel_spmd(nc, in_maps, list(range(8))).results
    out = np.zeros((T, HID), np.float32)
    for c in range(8):
        out += res[c]["out"].astype(np.float32)
    return out.reshape(2, 1024, HID).astype(np.float32)


# revision 9
# speedup vs baseline: 1.3074x; 1.0783x over previous
"""Trainium2 Bass kernel for MockMobGatedDeltaNetMoE (v11).

Sharding: head-parallel over H=8 heads, one head per NeuronCore.
Each core computes its head's full contribution; the host sums the 8
partial output projections.

v9: routing computed on host (rw/nsel/mask shipped); removes the
LDWEIGHTS-bound router matmuls and halves hidden-state DMA.
v10: merged single exp per (query-tile, expert) with one accum_out.

v11:
 - batch-interleaved emission: tb0,tb1 -> attention(b0) -> tb2,tb3 ->
   attention(b1) -> phase 4.  The b0 exp/DVE streams overlap the tb2/tb3
   projection matmuls; the b0->b1 transition bubble disappears.
 - one shared PSUM pool for phases 1+3: two [128,1024] tag groups
   ("big": projections/qm/scores, "acc": ptps/CT/AV) x bufs=2 = 8 banks.
 - phase-1 DMA as (hst,wq,wk) triples, then wv, tb1-hst, wg; q/k chains
   compute f0+f1 into one 2-bank psum; v and g get separate passes so the
   first matmul starts after ~2 transfers and never waits on wg.
 - dcd built in one tensor_scalar (dinv*rw fused); cmul kept f16 for a
   f16 CT transpose.
 - phase 4 emitted per token-tile (transpose+GEMM+store interleave).
"""

import numpy as np

import concourse.bass as bass
import concourse.bacc as bacc
import concourse.tile as tile
from concourse import mybir
from concourse.bass_utils import run_bass_kernel_spmd

F32 = mybir.dt.float32
F16 = mybir.dt.float16
ALU = mybir.AluOpType
ACTF = mybir.ActivationFunctionType

H, D, R, NE = 8, 256, 6, 4
HID, DV, T = 2048, 512, 2048
NB = 2
TB = T // NB
SCALE = 1.0 / 16.0


def _emit_phase1_tb(nc, tc, pools, tb, hst):
    """q/k/v/g projection chains for one 512-token block."""
    ps_pool, (wq_sb, wk_sb, wv_sb, wg_sb), (qT, kT, v_sb, g_sb) = pools
    t0 = tb * 512
    # q/k -> transposed [d-chunk, token]; f0+f1 share one 2-bank psum
    for wsb, dstT in ((wq_sb, qT), (wk_sb, kT)):
        ps = ps_pool.tile([128, 1024], F32, name="big", tag="big", bufs=2)
        for hc in range(16):
            nc.tensor.matmul(ps[:, 0:512], wsb[hc][:, 0:128], hst[hc][:],
                             start=(hc == 0), stop=(hc == 15))
            nc.tensor.matmul(ps[:, 512:1024], wsb[hc][:, 128:256], hst[hc][:],
                             start=(hc == 0), stop=(hc == 15))
        nc.scalar.copy(dstT[:, t0:t0 + 512], ps[:, 0:512])
        nc.scalar.copy(dstT[:, T + t0:T + t0 + 512], ps[:, 512:1024])
    # v then g (separate passes; wg arrives after wv/tb1-hst in DMA order)
    for wsb2, dst_sb, use_scalar in ((wv_sb, v_sb, True), (wg_sb, g_sb, False)):
        for half in range(2):
            ps = ps_pool.tile([128, 1024], F32, name="big", tag="big", bufs=2)
            for hc in range(16):
                nc.tensor.matmul(ps[:, 0:512],
                                 hst[hc][:, half * 256:half * 256 + 128],
                                 wsb2[hc][:], start=(hc == 0), stop=(hc == 15))
                nc.tensor.matmul(ps[:, 512:1024],
                                 hst[hc][:, half * 256 + 128:half * 256 + 256],
                                 wsb2[hc][:], start=(hc == 0), stop=(hc == 15))
            tt = tb * 4 + half * 2
            for s in range(2):
                dst = dst_sb[:, (tt + s) * DV:(tt + s + 1) * DV]
                src = ps[:, s * 512:(s + 1) * 512]
                if use_scalar:
                    nc.scalar.copy(dst, src)
                else:
                    nc.vector.tensor_copy(dst, src)


def _emit_silu(nc, p3, g_sb, tt_range):
    for tt in tt_range:
        sg = p3.tile([128, DV], F16, name="sg", tag="sg", bufs=2)
        nc.scalar.activation(sg[:], g_sb[:, tt * DV:(tt + 1) * DV], ACTF.Sigmoid)
        nc.vector.tensor_tensor(g_sb[:, tt * DV:(tt + 1) * DV], sg[:],
                                g_sb[:, tt * DV:(tt + 1) * DV], ALU.mult)


def _emit_attention(nc, tc, b, ps_pool, p3, tensors, ident, ident16):
    qT, kT, v_sb, g_sb, wqm_sb, rw_all, nsel, o_acc, mbk_d = tensors
    # --- kTm: shared set = plain kT slices; routed via host mask tiles ---
    ktm = [[kT[:, dc * T + b * TB:dc * T + (b + 1) * TB] for dc in range(2)]]
    for rs in range(1, 5):
        mb = p3.tile([128, TB], F16, name="mb", tag="mb", bufs=2)
        nc.sync.dma_start(
            out=mb[:],
            in_=mbk_d[:, ((rs - 1) * NB + b) * TB:((rs - 1) * NB + b + 1) * TB])
        pair = []
        for dc in range(2):
            kmt = p3.tile([128, TB], F16, name="ktm", tag=f"ktm{rs}{dc}", bufs=1)
            nc.vector.tensor_tensor(
                kmt[:], kT[:, dc * T + b * TB:dc * T + (b + 1) * TB],
                mb[:], ALU.mult)
            pair.append(kmt)
        ktm.append(pair)
    # --- nspur_b[r', :] = -sum_{masked k} v[k, :]  (rank-4) ---
    psn = ps_pool.tile([128, 1024], F32, name="big", tag="big", bufs=2)
    for kt in range(8):
        ktt = b * 8 + kt
        nc.tensor.matmul(psn[0:NE, 0:DV], nsel[:, ktt * NE:(ktt + 1) * NE],
                         v_sb[:, ktt * DV:(ktt + 1) * DV],
                         start=(kt == 0), stop=(kt == 7))
    nspur = p3.tile([NE, DV], F16, name="nspur", tag="nspur", bufs=2)
    nc.scalar.copy(nspur[:], psn[0:NE, 0:DV])
    # --- qmT for all r over this batch: [r][d2c] -> [128, TB] ---
    qmT = []
    for r in range(R):
        pair = []
        for d2c in range(2):
            qm = p3.tile([128, TB], F16, name="qmT", tag=f"qmT{r}{d2c}", bufs=1)
            psq = ps_pool.tile([128, 1024], F32, name="big", tag="big", bufs=2)
            for th in range(2):
                for dc in range(2):
                    nc.tensor.matmul(
                        psq[:, th * 512:(th + 1) * 512],
                        wqm_sb[:, dc * 1536 + r * 256 + d2c * 128:
                               dc * 1536 + r * 256 + d2c * 128 + 128],
                        qT[:, dc * T + b * TB + th * 512:
                           dc * T + b * TB + th * 512 + 512],
                        start=(dc == 0), stop=(dc == 1))
            nc.vector.tensor_copy(qm[:], psq[:])
            pair.append(qm)
        qmT.append(pair)
    # --- per query-tile: scores -> exp -> combine -> AV ---
    for qh in range(2):
        for qt in range(4):
            tt = b * 8 + qh * 4 + qt
            q0 = qh * 512 + qt * 128
            ptps = ps_pool.tile([128, 1024], F32, name="acc", tag="acc", bufs=2)
            cmul16 = p3.tile([128, R], F16, name="cmul16", tag="cmula", bufs=2)
            for r in range(R):
                krs = 0 if r < 2 else r - 1
                sps = ps_pool.tile([128, 1024], F32, name="big", tag="big", bufs=2)
                for kc in range(2):
                    for d2c in range(2):
                        nc.tensor.matmul(
                            sps[:, kc * 512:(kc + 1) * 512],
                            qmT[r][d2c][:, q0:q0 + 128],
                            ktm[krs][d2c][:, kc * 512:(kc + 1) * 512],
                            start=(d2c == 0), stop=(d2c == 1))
                es = p3.tile([128, 1024], F16, name="es", tag="es", bufs=2)
                dn = p3.tile([128, 1], F32, name="dn", tag="dn", bufs=4)
                nc.scalar.activation(es[:], sps[:], ACTF.Exp, scale=SCALE,
                                     accum_out=dn[:])
                dinv = p3.tile([128, 1], F32, name="adinv", tag="adinv", bufs=4)
                nc.vector.reciprocal(dinv[:], dn[:])
                dcd = p3.tile([128, 128], F16, name="dcd", tag="dcd", bufs=2)
                nc.vector.tensor_scalar(dcd[:], ident16[:], dinv[:],
                                        rw_all[:, tt * R + r:tt * R + r + 1],
                                        ALU.mult, ALU.mult)
                nc.vector.tensor_tensor(cmul16[:, r:r + 1], dinv[:],
                                        rw_all[:, tt * R + r:tt * R + r + 1],
                                        ALU.mult)
                # combine: start only on the first matmul touching each
                # PSUM bank (start clears has_written bank-wide).
                for kt in range(8):
                    nc.tensor.matmul(
                        ptps[:, kt * 128:(kt + 1) * 128],
                        es[:, kt * 128:(kt + 1) * 128],
                        dcd[:], start=(r == 0 and kt % 4 == 0),
                        stop=(r == R - 1))
            pts = p3.tile([128, 1024], F16, name="pts", tag="pts", bufs=2)
            nc.scalar.copy(pts[:], ptps[:])
            # CT: [4, 128] = cmul16[:, 2:6]^T (f16 transpose)
            psc = ps_pool.tile([128, 1024], F32, name="acc", tag="acc", bufs=2)
            nc.tensor.matmul(psc[0:NE, 0:128], cmul16[:, 2:R], ident16[:],
                             start=True, stop=True)
            ctb = p3.tile([NE, 128], F16, name="ctb", tag="ctb", bufs=2)
            nc.scalar.copy(ctb[:], psc[0:NE, 0:128])
            # AV + spur correction
            avp = ps_pool.tile([128, 1024], F32, name="acc", tag="acc", bufs=2)
            for kt in range(8):
                ktt = b * 8 + kt
                nc.tensor.matmul(avp[:, 0:DV], pts[:, kt * 128:(kt + 1) * 128],
                                 v_sb[:, ktt * DV:(ktt + 1) * DV],
                                 start=(kt == 0), stop=False)
            nc.tensor.matmul(avp[:, 0:DV], ctb[:], nspur[:], start=False, stop=True)
            nc.vector.tensor_copy(o_acc[:, tt * DV:(tt + 1) * DV], avp[:, 0:DV])


def _body(ctx, nc, tc, io):
    wq, wk, wv, wg, wqm, hst_d, rw_d, nsel_d, mbk_d, wo, out = io

    const = ctx.enter_context(tc.tile_pool(name="const", bufs=1))
    pers = ctx.enter_context(tc.tile_pool(name="pers", bufs=1))

    from concourse.masks import make_identity
    ident = const.tile([128, 128], F32, name="ident")
    make_identity(nc, ident)
    ident16 = const.tile([128, 128], F16, name="ident16")
    nc.vector.tensor_copy(ident16[:], ident[:])

    qT = pers.tile([128, 2 * T], F16, name="qT")         # [d-chunk, token]
    kT = pers.tile([128, 2 * T], F16, name="kT")         # [d-chunk, token]
    v_sb = pers.tile([128, 16 * DV], F16, name="v_sb")   # [token-tile, dv]
    g_sb = pers.tile([128, 16 * DV], F16, name="g_sb")   # [token-tile, dv]
    wqm_sb = pers.tile([128, 2 * 1536], F16, name="wqm_sb")
    rw_all = pers.tile([128, 16 * R], F32, name="rw_all")
    nsel = pers.tile([128, 16 * NE], F16, name="nsel")   # sel - 1 (0/-1)
    o_acc = pers.tile([128, 16 * DV], F16, name="o_acc")

    with tc.tile_pool(name="p1w", bufs=1) as p1w, \
         tc.tile_pool(name="p1", bufs=1) as p1, \
         tc.tile_pool(name="p3", bufs=1) as p3, \
         tc.tile_pool(name="ps", bufs=1, space="PSUM") as ps_pool:
        # ---- phase-1 DMA stream: (hst,wq,wk) triples, wv, tb1-hst, wg ----
        def hst_dmas(tb):
            tiles = []
            for hc in range(16):
                h3 = p1.tile([128, 512], F16, name="hst", tag="hst", bufs=18)
                nc.sync.dma_start(
                    out=h3[:], in_=hst_d[hc * 128:(hc + 1) * 128, tb * 512:tb * 512 + 512])
                tiles.append(h3)
            return tiles

        wq_sb, wk_sb, wv_sb, wg_sb = [], [], [], []
        hst0 = []
        for hc in range(16):
            h3 = p1.tile([128, 512], F16, name="hst", tag="hst", bufs=18)
            nc.sync.dma_start(out=h3[:], in_=hst_d[hc * 128:(hc + 1) * 128, 0:512])
            hst0.append(h3)
            for lst, src, wdt, nm in ((wq_sb, wq, 256, "wqsb"), (wk_sb, wk, 256, "wksb")):
                w1 = p1w.tile([128, wdt], F16, name=nm, tag=f"{nm}{hc}")
                nc.sync.dma_start(out=w1[:], in_=src[hc * 128:(hc + 1) * 128, :])
                lst.append(w1)
        for hc in range(16):
            w1 = p1w.tile([128, 512], F16, name="wvsb", tag=f"wvsb{hc}")
            nc.sync.dma_start(out=w1[:], in_=wv[hc * 128:(hc + 1) * 128, :])
            wv_sb.append(w1)
        hst1 = hst_dmas(1)
        for hc in range(16):
            w1 = p1w.tile([128, 512], F16, name="wgsb", tag=f"wgsb{hc}")
            nc.sync.dma_start(out=w1[:], in_=wg[hc * 128:(hc + 1) * 128, :])
            wg_sb.append(w1)
        nc.sync.dma_start(out=rw_all[:], in_=rw_d[:, :])
        nc.sync.dma_start(out=nsel[:], in_=nsel_d[:, :])
        for dc in range(2):
            nc.sync.dma_start(out=wqm_sb[:, dc * 1536:(dc + 1) * 1536],
                              in_=wqm[dc * 128:(dc + 1) * 128, :])

        pools = (ps_pool, (wq_sb, wk_sb, wv_sb, wg_sb), (qT, kT, v_sb, g_sb))
        attn_tensors = (qT, kT, v_sb, g_sb, wqm_sb, rw_all, nsel, o_acc, mbk_d)

        _emit_phase1_tb(nc, tc, pools, 0, hst0)
        _emit_phase1_tb(nc, tc, pools, 1, hst1)
        _emit_silu(nc, p3, g_sb, range(0, 8))
        _emit_attention(nc, tc, 0, ps_pool, p3, attn_tensors, ident, ident16)
        hst2 = hst_dmas(2)
        _emit_phase1_tb(nc, tc, pools, 2, hst2)
        hst3 = hst_dmas(3)
        _emit_phase1_tb(nc, tc, pools, 3, hst3)
        _emit_silu(nc, p3, g_sb, range(8, 16))
        _emit_attention(nc, tc, 1, ps_pool, p3, attn_tensors, ident, ident16)

    # ---------------- phase 4: gate, transpose, output projection ----------------
    with tc.tile_pool(name="p4", bufs=1) as p4, \
         tc.tile_pool(name="p4ps", bufs=1, space="PSUM") as p4ps:
        wo_sb = [p4.tile([128, HID], F16, name=f"wo_sb{i}", tag=f"wo{i}") for i in range(4)]
        for half in range(2):
            for i in range(4):
                nc.sync.dma_start(out=wo_sb[i][:, half * 1024:(half + 1) * 1024],
                                  in_=wo[i * 128:(i + 1) * 128, half * 1024:(half + 1) * 1024])
        for tt in range(16):
            xres = p4.tile([128, DV], F16, name="xres", tag="xres", bufs=3)
            nc.vector.tensor_tensor(xres[:], o_acc[:, tt * DV:(tt + 1) * DV],
                                    g_sb[:, tt * DV:(tt + 1) * DV], ALU.mult)
            xtt = p4.tile([128, DV], F16, name="xtt", tag="xtt", bufs=3)
            for dvc in range(4):
                pst = p4ps.tile([128, 128], F16, name="pst4", tag="pst4", bufs=2)
                nc.tensor.transpose(pst[:], xres[:, dvc * 128:(dvc + 1) * 128], ident16[:])
                nc.vector.tensor_copy(xtt[:, dvc * 128:(dvc + 1) * 128], pst[:])
            for hb in range(4):
                psf = p4ps.tile([128, 512], F32, name="psf", tag="psf", bufs=2)
                for dvc in range(4):
                    nc.tensor.matmul(psf[:], xtt[:, dvc * 128:(dvc + 1) * 128],
                                     wo_sb[dvc][:, hb * 512:(hb + 1) * 512],
                                     start=(dvc == 0), stop=(dvc == 3))
                ost = p4.tile([128, 512], F16, name="ost", tag="ost", bufs=4)
                if hb % 2 == 0:
                    nc.scalar.copy(ost[:], psf[:])
                else:
                    nc.vector.tensor_copy(ost[:], psf[:])
                nc.sync.dma_start(out=out[tt * 128:(tt + 1) * 128, hb * 512:(hb + 1) * 512],
                                  in_=ost[:])


_PROGRAM = None


def build_program():
    global _PROGRAM
    if _PROGRAM is not None:
        return _PROGRAM
    nc = bacc.Bacc("TRN2", target_bir_lowering=False, debug=False, num_devices=8)
    names = [("wq", [HID, D], F16), ("wk", [HID, D], F16),
             ("wv", [HID, DV], F16), ("wg", [HID, DV], F16),
             ("wqm", [D, D * R], F16),
             ("hst", [HID, T], F16),
             ("rw", [128, 16 * R], F32), ("nsel", [128, 16 * NE], F16),
             ("mbk", [128, NE * NB * TB], F16), ("wo", [DV, HID], F16)]
    io = [nc.dram_tensor(n, s, dt, kind="ExternalInput").ap() for n, s, dt in names]
    io.append(nc.dram_tensor("out", [T, HID], F16, kind="ExternalOutput").ap())
    with tile.TileContext(nc) as tc:
        from contextlib import ExitStack as ES
        with ES() as ctx:
            _body(ctx, nc, tc, io)
    nc.compile()
    _PROGRAM = nc
    return nc


def _host_routing(hs64, Wq64, Wgate64):
    """Per-head routing on host, matching the reference in f64.

    Returns rw [T, R] f32, msel [T, NE] f16 (1 active / 0 masked)."""
    S, K = 2, 2
    T_, _ = hs64.shape
    rw = np.zeros((T_, R), np.float64)
    rw[:, :S] = 0.25
    logits = hs64 @ (Wq64 @ Wgate64)          # [T, R-S]
    sc = np.exp(logits - logits.max(axis=-1, keepdims=True))
    sc /= sc.sum(axis=-1, keepdims=True)
    idx = np.argsort(-sc, axis=-1, kind="stable")[:, :K]   # top-2, ties -> low idx
    w = np.take_along_axis(sc, idx, axis=-1)
    w /= w.sum(axis=-1, keepdims=True)
    np.put_along_axis(rw[:, S:], idx, w * 0.5, axis=-1)
    msel = np.zeros((T_, NE), np.float64)
    np.put_along_axis(msel, idx, 1.0, axis=-1)
    return rw.astype(np.float32), msel.astype(np.float16)


def make_in_maps(hidden_states, Wq, Wk, Wv, Wq_exp, Wk_exp, Wgate, Wg, Wo):
    hs2 = np.asarray(hidden_states, np.float32).reshape(T, HID)
    hsT = np.ascontiguousarray(hs2.T.astype(np.float16))
    hs64 = hs2.astype(np.float64)
    Wq64 = np.asarray(Wq, np.float64)
    Wg64 = np.asarray(Wgate, np.float64)
    Wqe64 = np.asarray(Wq_exp, np.float64)
    Wke64 = np.asarray(Wk_exp, np.float64)
    in_maps = []
    for c in range(8):
        rw, msel = _host_routing(hs64, Wq64[:, c * D:(c + 1) * D], Wg64)
        rw_sb = np.ascontiguousarray(
            rw.reshape(16, 128, R).transpose(1, 0, 2).reshape(128, 16 * R))
        nsel_sb = np.ascontiguousarray(
            (msel - 1).reshape(16, 128, NE).transpose(1, 0, 2).reshape(128, 16 * NE)
        ).astype(np.float16)
        mbk = np.empty((128, NE * NB * TB), np.float16)
        for i in range(NE):
            for b in range(NB):
                mbk[:, (i * NB + b) * TB:(i * NB + b + 1) * TB] = \
                    msel[b * TB:(b + 1) * TB, i][None, :]
        wqm = np.empty((D, D * R), np.float16)
        for r in range(R):
            m = Wqe64[c][:, r * D:(r + 1) * D] @ Wke64[c][:, r * D:(r + 1) * D].T
            wqm[:, r * D:(r + 1) * D] = m.astype(np.float16)
        in_maps.append({
            "wq": np.asarray(Wq, np.float16)[:, c * D:(c + 1) * D].copy(),
            "wk": np.asarray(Wk, np.float16)[:, c * D:(c + 1) * D].copy(),
            "wv": np.asarray(Wv, np.float16)[:, c * DV:(c + 1) * DV].copy(),
            "wg": np.asarray(Wg, np.float16)[:, c * DV:(c + 1) * DV].copy(),
            "wqm": wqm,
            "hst": hsT,
            "rw": rw_sb, "nsel": nsel_sb, "mbk": mbk,
            "wo": np.asarray(Wo, np.float16)[c * DV:(c + 1) * DV, :].copy(),
        })
    return in_maps


def kernel(hidden_states, Wq, Wk, Wv, Wq_exp, Wk_exp, Wgate, Wg, Wo):
    nc = build_program()
    in_maps = make_in_maps(hidden_states, Wq, Wk, Wv, Wq_exp, Wk_exp, Wgate, Wg, Wo)
    res = run_bass_kernel_spmd(nc, in_maps, list(range(8))).results
    out = np.zeros((T, HID), np.float32)
    for c in range(8):
        out += res[c]["out"].astype(np.float32)
    return out.reshape(2, 1024, HID).astype(np.float32)
